# revision 1
# baseline (speedup 1.0000x reference)
"""Trainium2 Bass kernel for a 2-layer stacked bidirectional LSTM.

Problem (hardcoded): B=64, T=512, D=512, H=512, 2 BiLSTM layers,
Keras gate order [i, f, g, o], sigmoid recurrent activation, tanh cell
activation, merge_mode='concat'.

Sharding: 8 cores = 2 directions x 4 batch quarters (B'=16 per core).
Each core runs, for its (direction, quarter):
  phase 1: input projection zx0 = x @ W0 + b0      (big matmul, bf16)
  phase 2: layer-0 recurrence over T steps          (U0 stationary, bf16)
  phase 3: pairwise AllGather of layer-0 h sequences (fwd/bwd partners)
  phase 4: projection zx1 = [h0f | h0b] @ W1 + b1
  phase 5: layer-1 recurrence -> h1 sequence (f32 output)

Time-reversal for backward cores is handled with a per-core int flag and
register arithmetic on the recurrence's per-chunk DRAM indices, so all 8
cores run the identical SPMD program; all DRAM-resident sequences are in
TRUE time order.

Layouts (per core):
  xT   input [D, T*BQ] bf16, token = t*BQ + b  (true time)
  zxT  scratch [G/128=16, 128, T*BQ] bf16  (gate dim on partitions)
  h0T  scratch [T, H, BQ] bf16 (true time) -> AllGather -> [2, T, H, BQ]
  h1T  output [T, H, BQ] f32 (true time)

Recurrence state (SBUF): hT [128, H/128, BQ] bf16, c [128, H/128, BQ] f32.
Per step: z^T = U^T-accumulated PSUM (4 gate tiles, one PSUM bank each),
z-add with zx, sigmoid/tanh on ACT, cell update on DVE.
"""

import numpy as np
import ml_dtypes

import concourse.bass as bass
import concourse.mybir as mybir
import concourse.tile as tile
from concourse.bass import ds, ts
from concourse.bass_utils import run_bass_kernel_spmd
from concourse.expressions import smax

BF16 = mybir.dt.bfloat16
F32 = mybir.dt.float32
I32 = mybir.dt.int32
AF = mybir.ActivationFunctionType
ALU = mybir.AluOpType

# Problem dims (full size)
B_FULL, T_FULL, D_FULL, H_FULL = 64, 512, 512, 512
N_CORES = 8
N_Q = 4  # batch quarters; cores 2q (fwd) and 2q+1 (bwd) handle quarter q
CH = 8   # recurrence steps per For_i chunk

_MAXW = 1  # max sem-waits per instruction accepted by this walrus


def _fix_walrus_compat(nc):
    """Adapt Tile-emitted IR to the deployed walrus:
    - drop EVENT_SEMAPHORE_RANGE_CLEAR (InstISA) kernel-tail cleanup (only
      needed for NEFF re-execution with stale semaphores; each load starts
      from a clean state),
    - split instructions carrying more than _MAXW semaphore waits into
      leading single-wait NOPs (this walrus rejects multi-wait sync info).
    """
    n_split = n_drop = 0
    for bb in nc.main_func.blocks:
        insts = bb.instructions
        out = []
        for inst in insts:
            if isinstance(inst, mybir.InstISA):
                n_drop += 1
                continue
            si = inst.sync_info
            if si is not None and len(si.on_wait) > _MAXW:
                waits = list(si.on_wait)
                extra, keep = waits[:-_MAXW], waits[-_MAXW:]
                for w in extra:
                    nop = mybir.InstNoOp(
                        name=nc.get_next_instruction_name(), ins=[], outs=[])
                    nop.engine = inst.engine
                    nop.sync_info = mybir.SyncInfo(on_wait=[w], on_update=[])
                    out.append(nop)
                    n_split += 1
                inst.sync_info = mybir.SyncInfo(
                    on_wait=keep, on_update=list(si.on_update))
            out.append(inst)
        insts[:] = out
    return n_drop, n_split


def build_program(T=T_FULL, BQ=B_FULL // N_Q, D=D_FULL, H=H_FULL,
                  single_core=False):
    """Build the SPMD bass program (identical for all 8 cores).

    single_core=True replaces the AllGather with local DMA copies (for
    TimelineSim cost analysis only — data is wrong for the partner block).
    """
    G = 4 * H
    KD0 = D // 128          # k-chunks layer-0 projection
    KD1 = 2 * H // 128      # k-chunks layer-1 projection
    KH = H // 128           # k-chunks recurrence / h storage
    MC = G // 128           # m-chunks of gate dim
    MG = MC // 4            # m-chunks per gate
    TOK = T * BQ            # tokens per core
    TW = min(512, TOK)      # proj token-tile width
    NTOKC = TOK // TW       # proj token tiles
    NT = T // CH            # recurrence chunks
    assert T % CH == 0 and TOK % TW == 0 and D % 128 == 0 and H % 128 == 0

    nc = bass.Bass("TRN2", target_bir_lowering=False, debug=False,
                   num_devices=1 if single_core else N_CORES)

    # ---- I/O ----
    xT = nc.dram_tensor("xT", [D, TOK], BF16, kind="ExternalInput")
    flagf = nc.dram_tensor("flagf", [1, 1], F32, kind="ExternalInput")
    w0 = nc.dram_tensor("w0", [D, G], BF16, kind="ExternalInput")
    u0 = nc.dram_tensor("u0", [H, G], BF16, kind="ExternalInput")
    b0 = nc.dram_tensor("b0", [G], F32, kind="ExternalInput")
    w1 = nc.dram_tensor("w1", [2 * H, G], BF16, kind="ExternalInput")
    u1 = nc.dram_tensor("u1", [H, G], BF16, kind="ExternalInput")
    b1 = nc.dram_tensor("b1", [G], F32, kind="ExternalInput")
    flag = nc.dram_tensor("flag", [1, 1], I32, kind="ExternalInput")
    h1T = nc.dram_tensor("h1T", [T, H, BQ], F32, kind="ExternalOutput")

    groups = [[2 * q, 2 * q + 1] for q in range(N_Q)]

    with tile.TileContext(nc) as tc:
        # ---------------- persistent pools ----------------
        consts = tc.alloc_tile_pool(name="consts", bufs=1)
        dram = tc.alloc_tile_pool(name="dram", bufs=1, space="DRAM")

        # weights / biases resident in SBUF for the whole kernel
        w0_sb = consts.tile([128, KD0, G], BF16)
        nc.sync.dma_start(w0_sb, w0.ap().rearrange("(k p) g -> p k g", p=128))
        u0_sb = consts.tile([128, KH, G], BF16)
        nc.sync.dma_start(u0_sb, u0.ap().rearrange("(k p) g -> p k g", p=128))
        w1_sb = consts.tile([128, KD1, G], BF16)
        nc.sync.dma_start(w1_sb, w1.ap().rearrange("(k p) g -> p k g", p=128))
        u1_sb = consts.tile([128, KH, G], BF16)
        nc.sync.dma_start(u1_sb, u1.ap().rearrange("(k p) g -> p k g", p=128))
        b0_sb = consts.tile([128, MC], F32)
        nc.sync.dma_start(b0_sb, b0.ap().rearrange("(m p) -> p m", p=128))
        b1_sb = consts.tile([128, MC], F32)
        nc.sync.dma_start(b1_sb, b1.ap().rearrange("(m p) -> p m", p=128))
        flag_sb = consts.tile([1, 1], I32)
        nc.sync.dma_start(flag_sb, flag.ap())
        # broadcast flag as f32 per-partition scalar F (and 1-F) for the
        # data-driven time-flip selects
        F_bc = consts.tile([128, 1], F32)
        nc.sync.dma_start(
            F_bc,
            bass.AP(tensor=flagf, offset=0, ap=[[0, 128], [1, 1]]))
        Fc_bc = consts.tile([128, 1], F32)
        nc.vector.memset(Fc_bc, 1.0)
        nc.vector.tensor_tensor(Fc_bc, Fc_bc, F_bc, ALU.subtract)

        # DRAM scratch
        zxT0 = dram.tile([MC, 128, TOK], BF16)
        zxT1 = dram.tile([MC, 128, TOK], BF16)
        ag_in = dram.tile([T, H, BQ], BF16)
        ag_out = dram.tile([2 * T, H, BQ], BF16)

        fv = nc.values_load(flag_sb[0:1, 0:1], min_val=0, max_val=1)

        # ---------------- phase: projection ----------------
        def projection(src_kind, w_sb, b_sb, zxT, KD):
            with tc.tile_pool(name=f"proj_x_{src_kind}", bufs=2) as xpool, \
                 tc.tile_pool(name=f"proj_ps_{src_kind}", bufs=4,
                              space="PSUM") as pspool, \
                 tc.tile_pool(name=f"proj_ev_{src_kind}", bufs=4) as evpool:
                for tokc in range(NTOKC):
                    x_sb = xpool.tile([128, KD, TW], BF16, tag="xsb")
                    if src_kind == "xT":
                        nc.sync.dma_start(
                            x_sb,
                            xT.ap().rearrange("(k p) t -> p k t", p=128)[
                                :, :, ts(tokc, TW)])
                    else:
                        # ag_out [2T, H, BQ]: feature f = dir*H + 128*hk + p
                        # token tile tokc covers t in [tokc*TT, (tokc+1)*TT)
                        TT = TW // BQ
                        src = ag_out.rearrange(
                            "(dir t) (hk p) b -> p dir hk t b", dir=2, p=128)
                        for kk in range(KD):
                            nc.sync.dma_start(
                                x_sb[:, kk, :],
                                src[:, kk // (KD // 2), kk % (KD // 2),
                                    ts(tokc, TT), :])
                    for m in range(MC):
                        psum = pspool.tile([128, TW], F32, tag="pps")
                        for k in range(KD):
                            nc.tensor.matmul(
                                psum, w_sb[:, k, ts(m, 128)], x_sb[:, k, :],
                                start=(k == 0), stop=(k == KD - 1))
                        ev = evpool.tile([128, TW], BF16, tag="pev")
                        nc.scalar.activation(ev, psum, AF.Identity,
                                             bias=b_sb[:, m:m + 1])
                        nc.sync.dma_start(zxT[m][:, ts(tokc, TW)], ev)

        # ---------------- phase: recurrence ----------------
        def recurrence(u_sb, zxT, layer):
            state = tc.alloc_tile_pool(name=f"state{layer}", bufs=1)
            h_sb = state.tile([128, KH, BQ], BF16, name=f"hsb{layer}")
            c_sb = state.tile([128, KH, BQ], F32, name=f"csb{layer}")
            nc.vector.memset(h_sb, 0.0)
            nc.vector.memset(c_sb, 0.0)

            zx_r = zxT.rearrange("m p tok -> p m tok")
            if layer == 0:
                hT_r = ag_in.rearrange("t (k p) b -> p t k b", p=128)
            else:
                hT_r = h1T.ap().rearrange("t (k p) b -> p t k b", p=128)

            with tc.tile_pool(name=f"zx{layer}", bufs=2) as zxpool, \
                 tc.tile_pool(name=f"hck{layer}", bufs=2) as hckpool, \
                 tc.tile_pool(name=f"zg{layer}", bufs=2) as zgpool, \
                 tc.tile_pool(name=f"ps_i{layer}", bufs=2,
                              space="PSUM") as ps_i, \
                 tc.tile_pool(name=f"ps_f{layer}", bufs=2,
                              space="PSUM") as ps_f, \
                 tc.tile_pool(name=f"ps_g{layer}", bufs=2,
                              space="PSUM") as ps_g, \
                 tc.tile_pool(name=f"ps_o{layer}", bufs=2,
                              space="PSUM") as ps_o:
                gate_pools = [ps_i, ps_f, ps_g, ps_o]
                with tc.For_i(0, NT, 1) as cc:
                    # true-time chunk index (flipped for bwd cores):
                    # fwd: cc ; bwd: NT-1-cc   via smax(cc - K, K - cc)
                    K = fv * (NT - 1)
                    tcix = nc.s_assert_within(smax(cc - K, K - cc), 0, NT - 1)
                    zx_sb = zxpool.tile([128, MC, CH * BQ], BF16, tag="zxc")
                    nc.sync.dma_start(
                        zx_sb, zx_r[:, :, ds(tcix * (CH * BQ), CH * BQ)])
                    hdt = F32 if layer == 1 else BF16
                    h_ck = hckpool.tile([128, CH, KH, BQ], hdt, tag="hck")
                    h_ckT = hckpool.tile([128, CH, KH, BQ], hdt, tag="hckT")
                    for j in range(CH):
                        psg = [gate_pools[g].tile([128, MG, BQ], F32,
                                                  tag=f"psg{g}",
                                                  name=f"psg{g}")
                               for g in range(4)]
                        z16 = zgpool.tile([128, MC, BQ], F32, tag="z16")
                        g16 = zgpool.tile([128, MC, BQ], F32, tag="g16")
                        # local-time zx slice via data-driven select:
                        # zxj = (1-F)*zx[j] + F*zx[CH-1-j]
                        zxj = zgpool.tile([128, MC, BQ], F32, tag="zxj")
                        tmpz = zgpool.tile([128, MC, BQ], F32, tag="tmpz")
                        nc.vector.tensor_scalar_mul(
                            tmpz, zx_sb[:, :, ts(CH - 1 - j, BQ)], F_bc)
                        nc.vector.scalar_tensor_tensor(
                            zxj, zx_sb[:, :, ts(j, BQ)], Fc_bc, tmpz,
                            ALU.mult, ALU.add)
                        for g in range(4):
                            for mm in range(MG):
                                m = g * MG + mm
                                for k in range(KH):
                                    nc.tensor.matmul(
                                        psg[g][:, mm, :],
                                        u_sb[:, k, ts(m, 128)],
                                        h_sb[:, k, :],
                                        start=(k == 0), stop=(k == KH - 1))
                            nc.vector.tensor_tensor(
                                z16[:, ts(g, MG), :], psg[g],
                                zxj[:, ts(g, MG), :], ALU.add)
                        # activations: sigmoid(i,f), tanh(g), sigmoid(o)
                        nc.scalar.activation(g16[:, 0:2 * MG, :],
                                             z16[:, 0:2 * MG, :], AF.Sigmoid)
                        nc.scalar.activation(g16[:, 2 * MG:3 * MG, :],
                                             z16[:, 2 * MG:3 * MG, :], AF.Tanh)
                        nc.scalar.activation(g16[:, 3 * MG:4 * MG, :],
                                             z16[:, 3 * MG:4 * MG, :],
                                             AF.Sigmoid)
                        ig = zgpool.tile([128, MG, BQ], F32, tag="ig")
                        fc = zgpool.tile([128, MG, BQ], F32, tag="fc")
                        nc.vector.tensor_tensor(ig, g16[:, 0:MG, :],
                                                g16[:, 2 * MG:3 * MG, :],
                                                ALU.mult)
                        nc.vector.tensor_tensor(fc, g16[:, MG:2 * MG, :],
                                                c_sb, ALU.mult)
                        nc.vector.tensor_tensor(c_sb, ig, fc, ALU.add)
                        th = zgpool.tile([128, MG, BQ], F32, tag="th")
                        nc.scalar.activation(th, c_sb, AF.Tanh)
                        # h = o * tanh(c): bf16 state copy for next matmul
                        nc.vector.tensor_tensor(h_sb, g16[:, 3 * MG:, :], th,
                                                ALU.mult)
                        # store into h chunk at local slot (off crit path)
                        if layer == 0:
                            nc.scalar.copy(h_ck[:, j, :, :], h_sb)
                        else:
                            nc.vector.tensor_tensor(
                                h_ck[:, j, :, :],
                                g16[:, 3 * MG:, :], th, ALU.mult)
                    # reorder chunk local->true time on gpsimd (idle engine):
                    # h_ckT[s] = (1-F)*h_ck[s] + F*h_ck[CH-1-s]
                    for s in range(CH):
                        tsel = zgpool.tile([128, KH, BQ], hdt, tag="tsel")
                        nc.vector.tensor_scalar_mul(
                            tsel, h_ck[:, CH - 1 - s, :, :], F_bc)
                        nc.vector.scalar_tensor_tensor(
                            h_ckT[:, s, :, :], h_ck[:, s, :, :], Fc_bc, tsel,
                            ALU.mult, ALU.add)
                    nc.sync.dma_start(
                        hT_r[:, ds(tcix * CH, CH), :, :], h_ckT)
            state.release()

        projection("xT", w0_sb, b0_sb, zxT0, KD0)
        recurrence(u0_sb, zxT0, 0)
        if single_core:
            ag_v = ag_out.rearrange("(dir t) h b -> dir t h b", dir=2)
            nc.sync.dma_start(ag_v[0], ag_in)
            nc.sync.dma_start(ag_v[1], ag_in)
        else:
            nc.gpsimd.collective_compute(
                "AllGather", ALU.bypass, replica_groups=groups,
                ins=[ag_in.opt()], outs=[ag_out.opt()])
        projection("ag", w1_sb, b1_sb, zxT1, KD1)
        recurrence(u1_sb, zxT1, 1)

        dram.release()
        consts.release()

    _fix_walrus_compat(nc)
    return nc


def _prep_core_inputs(x, W0f, U0f, b0f, W0b, U0b, b0b,
                      W1f, U1f, b1f, W1b, U1b, b1b, T, BQ):
    """Host-side sharding: returns list of 8 input dicts (core = 2q+dir)."""
    bf = ml_dtypes.bfloat16
    in_maps = []
    Wd = {0: (W0f, U0f, b0f, W1f, U1f, b1f),
          1: (W0b, U0b, b0b, W1b, U1b, b1b)}
    for q in range(N_Q):
        xq = x[q * BQ:(q + 1) * BQ]              # [BQ, T, D]
        # xT [D, T*BQ], tok = t*BQ + b, true time for both directions
        xT = np.ascontiguousarray(
            xq.transpose(2, 1, 0).reshape(x.shape[2], T * BQ)).astype(bf)
        for d in range(2):
            W0, U0, b0, W1, U1, b1 = Wd[d]
            in_maps.append({
                "xT": xT,
                "w0": W0.astype(bf), "u0": U0.astype(bf),
                "b0": b0.astype(np.float32),
                "w1": W1.astype(bf), "u1": U1.astype(bf),
                "b1": b1.astype(np.float32),
                "flag": np.array([[d]], dtype=np.int32),
                "flagf": np.array([[d]], dtype=np.float32),
            })
    return in_maps


def kernel(x, W0f, U0f, b0f, W0b, U0b, b0b,
           W1f, U1f, b1f, W1b, U1b, b1b):
    x = np.asarray(x, dtype=np.float32)
    B, T, D = x.shape
    H = U0f.shape[0]
    BQ = B // N_Q
    nc = build_program(T=T, BQ=BQ, D=D, H=H)
    in_maps = _prep_core_inputs(
        np.asarray(x), np.asarray(W0f), np.asarray(U0f), np.asarray(b0f),
        np.asarray(W0b), np.asarray(U0b), np.asarray(b0b),
        np.asarray(W1f), np.asarray(U1f), np.asarray(b1f),
        np.asarray(W1b), np.asarray(U1b), np.asarray(b1b), T, BQ)
    res = run_bass_kernel_spmd(nc, in_maps, list(range(N_CORES)))
    out = np.empty((B, T, 2 * H), dtype=np.float32)
    for q in range(N_Q):
        for d in range(2):
            h1T = res.results[2 * q + d]["h1T"]   # [T, H, BQ] true time
            out[q * BQ:(q + 1) * BQ, :, d * H:(d + 1) * H] = \
                h1T.transpose(2, 0, 1)
    return out

